# revision 25
# baseline (speedup 1.0000x reference)
"""Trainium2 Bass kernel for nn_DKL_45810121179236 (retrieval_knn).

Reference computation:
    C = cos_sim_matrix(ex, ey)            # [8192, 8192], D=256
    out1 = -sum(exp(c1)*c1), c1 = logN(1 - rowmax(C))
    out2 = -sum(exp(c2)*c2), c2 = logN(1 - colmax(C))

Sharding: ex rows split across 8 NeuronCores (1024 rows each); each core
computes its [1024, 8192] tile of C against the full ey, producing exact
local row-maxes and partial col-maxes. Host combines: concat row-maxes,
8-way elementwise max of col-max partials, then the two O(N) entropy sums.

Per-core pipeline (engines in parens):
  load:   1MB batches, 1024 rows as [128, (8, 256)] (sync HWDGE)
  norm:   per 256-slice sum-of-squares (ACT Square+accum), sqrt (ACT),
          reciprocal (DVE), scale+cast bf16 (ACT Copy w/ scale AP)
  transp: PE transpose 128x128 into PSUM, 4 per bank, copy out (DVE)
  mm:     [128, 1024] PSUM groups; 2x(N=512) x 2 K-chunk bf16 matmuls (PE)
  reduce: PSUM->SBUF bf16 copy (ACT); col-max acc TT-max (DVE, 2x mode);
          row-max chain TT-max (DVE)
  fold:   PE-transpose col-max acc, 3-D reduce_max (DVE)
"""

import sys

sys.path.insert(0, "/opt/trn_rl_repo")

import copy
from contextlib import ExitStack

import numpy as np

import concourse.bass as bass
import concourse.tile as tile
from concourse import mybir
from concourse import bass_utils
from concourse.masks import make_identity

N_CORES = 8
N = 8192  # ey rows (and total ex rows)
D = 256  # embedding dim
XR = N // N_CORES  # ex rows per core (1024)
NT_X = XR // 128  # 8 x-tiles per core
NT_Y = N // 128  # 64 y-tiles
NB_Y = 8  # y load batches (1024 rows each)
JG = 1024  # j-group width = 2 PSUM banks
NG = N // JG  # 8 j-groups per x-tile

F32 = mybir.dt.float32
BF16 = mybir.dt.bfloat16
AF = mybir.ActivationFunctionType
ALU = mybir.AluOpType
AX = mybir.AxisListType

SIGMA = 0.3


def _split_multi_waits(nc, max_waits=1):
    """The walrus build in this container rejects instructions carrying more
    than one sync wait. Move excess waits onto preceding same-engine NOPs
    (waits on one engine are sequential, so semantics are unchanged)."""
    n_split = 0
    for function in nc.m.functions:
        new_blocks = []
        for block in function.blocks:
            new_insts = []
            for inst in block.instructions:
                si = inst.sync_info
                if si is not None and si.on_wait and len(si.on_wait) > max_waits:
                    waits = list(si.on_wait)
                    n_split += 1
                    head, rest = waits[:-max_waits], waits[-max_waits:]
                    for ci in range(0, len(head), max_waits):
                        new_insts.append(
                            mybir.InstNoOp(
                                name=f"{inst.name}-ws{ci}",
                                engine=inst.engine,
                                sync_info=mybir.SyncInfo(
                                    on_wait=head[ci : ci + max_waits], on_update=[]
                                ),
                            )
                        )
                    inst = copy.replace(
                        inst,
                        sync_info=mybir.SyncInfo(
                            on_wait=rest, on_update=list(si.on_update)
                        ),
                    )
                new_insts.append(inst)
            new_blocks.append(copy.replace(block, instructions=new_insts))
        function.blocks.clear()
        for b in new_blocks:
            function.blocks.append(b)
    return n_split


def _emit_prep_batch(nc, pools, src_pqd, b, nq, tT_hi, tT_lo, ident, rns_out=None):
    """Load nq*128 rows, normalize, cast bf16, PE-transpose into tT_hi/lo
    columns [b*128*nq, ...). src_pqd is a [p, q, d] DRAM view of this batch.
    With rns_out, the scale step is skipped (folded downstream) and the
    reciprocal norms are stored there instead."""
    raw = pools["raw"].tile([128, nq * D], F32, tag="raw")
    nc.sync.dma_start(raw[:].rearrange("p (q d) -> p q d", q=nq), src_pqd)

    nsq = pools["sc"].tile([128, nq], F32, tag="sc")
    sq = pools["sq"].tile([128, D], F32, tag="sq")
    for q in range(nq):
        nc.vector.scalar_tensor_tensor(
            sq[:],
            raw[:, q * D : (q + 1) * D],
            1.0,
            raw[:, q * D : (q + 1) * D],
            ALU.mult,
            ALU.mult,
            accum_out=nsq[:, q : q + 1],
        )
    nrm = pools["sc"].tile([128, nq], F32, tag="sc")
    nc.scalar.activation(nrm[:], nsq[:], AF.Sqrt)
    rns = rns_out if rns_out is not None else pools["sc"].tile(
        [128, nq], F32, tag="sc"
    )
    nc.vector.reciprocal(rns[:], nrm[:])

    ybf = pools["bf"].tile([128, nq * D], BF16, tag="bf")
    for q in range(nq):
        if rns_out is None:
            # scale+cast on ACT: out = Copy(in * rns[q])
            nc.scalar.activation(
                ybf[:, q * D : (q + 1) * D],
                raw[:, q * D : (q + 1) * D],
                AF.Copy,
                scale=rns[:, q : q + 1],
            )
        else:
            # x side: plain cast; 1/||x_i|| folds into the PSUM->SBUF copies
            nc.scalar.activation(
                ybf[:, q * D : (q + 1) * D], raw[:, q * D : (q + 1) * D], AF.Copy
            )

    # PE transposes: 4 per [128, 512] psum tile, then one copy per tile
    for half, tT in ((0, tT_hi), (1, tT_lo)):
        for qq in range(0, nq, 4):
            ps = pools["pps"].tile([128, 512], BF16, tag="pps")
            for q in range(qq, min(qq + 4, nq)):
                nc.tensor.transpose(
                    ps[:, (q - qq) * 128 : (q - qq + 1) * 128],
                    ybf[:, q * D + half * 128 : q * D + half * 128 + 128],
                    ident[:],
                )
            w = (min(qq + 4, nq) - qq) * 128
            col0 = b * nq * 128 + qq * 128
            nc.vector.tensor_copy(tT[:, col0 : col0 + w], ps[:, 0:w])


def _build():
    nc = bass.Bass("TRN2", target_bir_lowering=False, debug=False, num_devices=1)
    ex = nc.dram_tensor("ex_sh", [XR, D], F32, kind="ExternalInput").ap()
    ey = nc.dram_tensor("ey", [N, D], F32, kind="ExternalInput").ap()
    rowmax_o = nc.dram_tensor("rowmax", [XR], F32, kind="ExternalOutput").ap()
    colmax_o = nc.dram_tensor("colmax", [N], F32, kind="ExternalOutput").ap()

    with tile.TileContext(nc) as tc:
        with ExitStack() as ctx:
            ep = ctx.enter_context

            persist = ep(tc.tile_pool(name="persist", bufs=1))
            yT_hi = persist.tile([128, N], BF16, tag="yT_hi")
            yT_lo = persist.tile([128, N], BF16, tag="yT_lo")
            xT_hi = persist.tile([128, XR], BF16, tag="xT_hi")
            xT_lo = persist.tile([128, XR], BF16, tag="xT_lo")
            colacc = persist.tile([128, N], BF16, tag="colacc")
            rowacc_all = persist.tile([128, N], BF16, tag="rowacc_all")
            rowmax_sb = persist.tile([128, NT_X], F32, tag="rowmax_sb")
            colmax_sb = persist.tile([128, NT_Y], F32, tag="colmax_sb")
            rx_sb = persist.tile([128, NT_X], F32, tag="rx_sb")
            ident_bf = persist.tile([128, 128], BF16, tag="ident_bf")
            ident_f32 = persist.tile([128, 128], F32, tag="ident_f32")
            make_identity(nc, ident_bf[:])
            make_identity(nc, ident_f32[:])

            pools = {
                "raw": ep(tc.tile_pool(name="raw", bufs=3)),
                "sq": ep(tc.tile_pool(name="sq", bufs=2)),
                "sc": ep(tc.tile_pool(name="sc", bufs=9)),
                "bf": ep(tc.tile_pool(name="bf", bufs=3)),
                "pps": ep(tc.tile_pool(name="pps", bufs=2, space="PSUM")),
            }
            mm_pool = ep(tc.tile_pool(name="mm", bufs=3, space="PSUM"))
            csb_pool = ep(tc.tile_pool(name="csb", bufs=4))
            out_pool = ep(tc.tile_pool(name="out", bufs=2))

            # ---- prep: x (one batch), then y (8 batches) ----
            xv = ex.rearrange("(q p) d -> p q d", p=128)
            _emit_prep_batch(nc, pools, xv, 0, NT_X, xT_hi, xT_lo, ident_bf)
            # ---- y-prep interleaved with the matmul sweep (j-outer) ----
            # After each y batch (one j-group) is ready, immediately run all
            # 8 x-tiles' matmul groups against it: the PE gets dense work
            # early and stays at the un-throttled clock.
            yv = ey.rearrange("(b q p) d -> b p q d", p=128, q=8)
            for b in range(NB_Y):
                _emit_prep_batch(nc, pools, yv[b], b, 8, yT_hi, yT_lo, ident_bf)
                g = b  # j-group == y batch (JG == 1024 rows per batch)
                for xt in range(NT_X):
                    xs = slice(xt * 128, (xt + 1) * 128)
                    ps = mm_pool.tile([128, JG], F32, tag="mm")
                    for c in range(JG // 512):
                        j0 = g * JG + c * 512
                        pslice = ps[:, c * 512 : (c + 1) * 512]
                        nc.tensor.matmul(
                            pslice,
                            xT_hi[:, xs],
                            yT_hi[:, j0 : j0 + 512],
                            start=True,
                            stop=False,
                        )
                        nc.tensor.matmul(
                            pslice,
                            xT_lo[:, xs],
                            yT_lo[:, j0 : j0 + 512],
                            start=False,
                            stop=True,
                        )
                    c_sb = csb_pool.tile([128, JG], BF16, tag="csb")
                    nc.scalar.activation(c_sb[:], ps[:], AF.Copy)
                    # col-max accumulate across x-tiles
                    acc_slice = colacc[:, g * JG : (g + 1) * JG]
                    if xt == 0:
                        nc.vector.tensor_copy(acc_slice, c_sb[:])
                    else:
                        nc.vector.tensor_max(acc_slice, acc_slice, c_sb[:])
                    # row-max accumulate across j-groups
                    row_slice = rowacc_all[:, xt * JG : (xt + 1) * JG]
                    if g == 0:
                        nc.vector.tensor_copy(row_slice, c_sb[:])
                    else:
                        nc.vector.tensor_max(row_slice, row_slice, c_sb[:])

            for xt in range(NT_X):
                nc.vector.reduce_max(
                    rowmax_sb[:, xt : xt + 1],
                    rowacc_all[:, xt * JG : (xt + 1) * JG],
                    axis=AX.X,
                )

            # ---- col-max partition fold ----
            for fg in range(NT_Y // 4):
                ps = pools["pps"].tile([128, 512], BF16, tag="pps")
                for k in range(4):
                    cch = fg * 4 + k
                    nc.tensor.transpose(
                        ps[:, k * 128 : (k + 1) * 128],
                        colacc[:, cch * 128 : (cch + 1) * 128],
                        ident_bf[:],
                    )
                nc.vector.reduce_max(
                    colmax_sb[:, fg * 4 : (fg + 1) * 4],
                    ps[:].rearrange("p (k q) -> p k q", k=4),
                    axis=AX.X,
                )

            # ---- outputs: transpose on PE so DMA writes are contiguous ----
            pso = pools["pps"].tile([128, 128], F32, tag="pps")
            # rowmax [128, 8] -> [8, 128]
            nc.tensor.transpose(pso[0:8, 0:128], rowmax_sb[:], ident_f32[:])
            rout = out_pool.tile([128, 128], F32, tag="out")
            nc.vector.tensor_copy(rout[0:8, 0:128], pso[0:8, 0:128])
            nc.sync.dma_start(rowmax_o.rearrange("(t p) -> t p", p=128), rout[0:8, :])
            # colmax [128, 64] -> [64, 128]
            pso2 = pools["pps"].tile([128, 128], F32, tag="pps")
            nc.tensor.transpose(pso2[0:64, 0:128], colmax_sb[:], ident_f32[:])
            cout = out_pool.tile([128, 128], F32, tag="out")
            nc.vector.tensor_copy(cout[0:64, 0:128], pso2[0:64, 0:128])
            nc.sync.dma_start(colmax_o.rearrange("(c p) -> c p", p=128), cout[0:64, :])

    _split_multi_waits(nc)
    return nc


_NC_CACHE = []


def _get_nc():
    if not _NC_CACHE:
        _NC_CACHE.append(_build())
    return _NC_CACHE[0]


def run_device(ex, ey, trace=False):
    """Run the SPMD kernel; returns (rowmax [N], colmax [N], results obj)."""
    nc = _get_nc()
    in_maps = [
        {"ex_sh": np.ascontiguousarray(ex[k * XR : (k + 1) * XR]), "ey": ey}
        for k in range(N_CORES)
    ]
    res = bass_utils.run_bass_kernel_spmd(
        nc, in_maps, core_ids=list(range(N_CORES)), trace=trace
    )
    rowmax = np.concatenate([res.results[k]["rowmax"] for k in range(N_CORES)])
    colmax = np.max(
        np.stack([res.results[k]["colmax"] for k in range(N_CORES)]), axis=0
    )
    return rowmax, colmax, res


def _entropy(m):
    # -sum(exp(c)*c), c = logprob_Normal(1,SIGMA)(1 - m); accumulate in f64
    z = -m.astype(np.float64) / SIGMA
    c = -0.5 * z * z - np.log(SIGMA) - 0.5 * np.log(2.0 * np.pi)
    return -np.sum(np.exp(c) * c)


def kernel(ex, ey):
    ex = np.ascontiguousarray(np.asarray(ex), dtype=np.float32)
    ey = np.ascontiguousarray(np.asarray(ey), dtype=np.float32)
    rowmax, colmax, _ = run_device(ex, ey)
    out1 = np.float32(_entropy(rowmax))
    out2 = np.float32(_entropy(colmax))
    return (np.asarray(out1, dtype=np.float32), np.asarray(out2, dtype=np.float32))


# revision 26
# speedup vs baseline: 1.0366x; 1.0366x over previous
"""Trainium2 Bass kernel for nn_DKL_45810121179236 (retrieval_knn).

Reference computation:
    C = cos_sim_matrix(ex, ey)            # [8192, 8192], D=256
    out1 = -sum(exp(c1)*c1), c1 = logN(1 - rowmax(C))
    out2 = -sum(exp(c2)*c2), c2 = logN(1 - colmax(C))

Sharding: ex rows split across 8 NeuronCores (1024 rows each); each core
computes its [1024, 8192] tile of C against the full ey, producing exact
local row-maxes and partial col-maxes. Host combines: concat row-maxes,
8-way elementwise max of col-max partials, then the two O(N) entropy sums.

Per-core pipeline (engines in parens):
  load:   1MB batches, 1024 rows as [128, (8, 256)] (sync HWDGE)
  norm:   per 256-slice sum-of-squares (ACT Square+accum), sqrt (ACT),
          reciprocal (DVE), scale+cast bf16 (ACT Copy w/ scale AP)
  transp: PE transpose 128x128 into PSUM, 4 per bank, copy out (DVE)
  mm:     [128, 1024] PSUM groups; 2x(N=512) x 2 K-chunk bf16 matmuls (PE)
  reduce: PSUM->SBUF bf16 copy (ACT); col-max acc TT-max (DVE, 2x mode);
          row-max chain TT-max (DVE)
  fold:   PE-transpose col-max acc, 3-D reduce_max (DVE)
"""

import sys

sys.path.insert(0, "/opt/trn_rl_repo")

import copy
from contextlib import ExitStack

import numpy as np

import concourse.bass as bass
import concourse.tile as tile
from concourse import mybir
from concourse import bass_utils
from concourse.masks import make_identity

N_CORES = 8
N = 8192  # ey rows (and total ex rows)
D = 256  # embedding dim
XR = N // N_CORES  # ex rows per core (1024)
NT_X = XR // 128  # 8 x-tiles per core
NT_Y = N // 128  # 64 y-tiles
NB_Y = 8  # y load batches (1024 rows each)
JG = 1024  # j-group width = 2 PSUM banks
NG = N // JG  # 8 j-groups per x-tile

F32 = mybir.dt.float32
BF16 = mybir.dt.bfloat16
AF = mybir.ActivationFunctionType
ALU = mybir.AluOpType
AX = mybir.AxisListType

SIGMA = 0.3


def _split_multi_waits(nc, max_waits=1):
    """The walrus build in this container rejects instructions carrying more
    than one sync wait. Move excess waits onto preceding same-engine NOPs
    (waits on one engine are sequential, so semantics are unchanged)."""
    n_split = 0
    for function in nc.m.functions:
        new_blocks = []
        for block in function.blocks:
            new_insts = []
            for inst in block.instructions:
                si = inst.sync_info
                if si is not None and si.on_wait and len(si.on_wait) > max_waits:
                    waits = list(si.on_wait)
                    n_split += 1
                    head, rest = waits[:-max_waits], waits[-max_waits:]
                    for ci in range(0, len(head), max_waits):
                        new_insts.append(
                            mybir.InstNoOp(
                                name=f"{inst.name}-ws{ci}",
                                engine=inst.engine,
                                sync_info=mybir.SyncInfo(
                                    on_wait=head[ci : ci + max_waits], on_update=[]
                                ),
                            )
                        )
                    inst = copy.replace(
                        inst,
                        sync_info=mybir.SyncInfo(
                            on_wait=rest, on_update=list(si.on_update)
                        ),
                    )
                new_insts.append(inst)
            new_blocks.append(copy.replace(block, instructions=new_insts))
        function.blocks.clear()
        for b in new_blocks:
            function.blocks.append(b)
    return n_split


def _emit_prep_batch(nc, pools, src_pqd, b, nq, tT_hi, tT_lo, ident, rns_out=None):
    """Load nq*128 rows, normalize, cast bf16, PE-transpose into tT_hi/lo
    columns [b*128*nq, ...). src_pqd is a [p, q, d] DRAM view of this batch.
    With rns_out, the scale step is skipped (folded downstream) and the
    reciprocal norms are stored there instead."""
    raw = pools["raw"].tile([128, nq * D], F32, tag="raw")
    nc.sync.dma_start(raw[:].rearrange("p (q d) -> p q d", q=nq), src_pqd)

    nsq = pools["sc"].tile([128, nq], F32, tag="sc")
    sq = pools["sq"].tile([128, D], F32, tag="sq")
    for q in range(nq):
        nc.vector.scalar_tensor_tensor(
            sq[:],
            raw[:, q * D : (q + 1) * D],
            1.0,
            raw[:, q * D : (q + 1) * D],
            ALU.mult,
            ALU.mult,
            accum_out=nsq[:, q : q + 1],
        )
    nrm = pools["sc"].tile([128, nq], F32, tag="sc")
    nc.scalar.activation(nrm[:], nsq[:], AF.Sqrt)
    rns = rns_out if rns_out is not None else pools["sc"].tile(
        [128, nq], F32, tag="sc"
    )
    nc.vector.reciprocal(rns[:], nrm[:])

    ybf = pools["bf"].tile([128, nq * D], BF16, tag="bf")
    for q in range(nq):
        if rns_out is None:
            # scale+cast on ACT: out = Copy(in * rns[q])
            nc.scalar.activation(
                ybf[:, q * D : (q + 1) * D],
                raw[:, q * D : (q + 1) * D],
                AF.Copy,
                scale=rns[:, q : q + 1],
            )
        else:
            # x side: plain cast; 1/||x_i|| folds into the PSUM->SBUF copies
            nc.scalar.activation(
                ybf[:, q * D : (q + 1) * D], raw[:, q * D : (q + 1) * D], AF.Copy
            )

    # PE transposes: 4 per [128, 512] psum tile, then one copy per tile
    for half, tT in ((0, tT_hi), (1, tT_lo)):
        for qq in range(0, nq, 4):
            ps = pools["pps"].tile([128, 512], BF16, tag="pps")
            for q in range(qq, min(qq + 4, nq)):
                nc.tensor.transpose(
                    ps[:, (q - qq) * 128 : (q - qq + 1) * 128],
                    ybf[:, q * D + half * 128 : q * D + half * 128 + 128],
                    ident[:],
                )
            w = (min(qq + 4, nq) - qq) * 128
            col0 = b * nq * 128 + qq * 128
            nc.vector.tensor_copy(tT[:, col0 : col0 + w], ps[:, 0:w])


def _build():
    nc = bass.Bass("TRN2", target_bir_lowering=False, debug=False, num_devices=1)
    ex = nc.dram_tensor("ex_sh", [XR, D], F32, kind="ExternalInput").ap()
    ey = nc.dram_tensor("ey", [N, D], F32, kind="ExternalInput").ap()
    rowmax_o = nc.dram_tensor("rowmax", [XR], F32, kind="ExternalOutput").ap()
    colmax_o = nc.dram_tensor("colmax", [N], F32, kind="ExternalOutput").ap()

    with tile.TileContext(nc) as tc:
        with ExitStack() as ctx:
            ep = ctx.enter_context

            persist = ep(tc.tile_pool(name="persist", bufs=1))
            yT_hi = persist.tile([128, N], BF16, tag="yT_hi")
            yT_lo = persist.tile([128, N], BF16, tag="yT_lo")
            xT_hi = persist.tile([128, XR], BF16, tag="xT_hi")
            xT_lo = persist.tile([128, XR], BF16, tag="xT_lo")
            colacc = persist.tile([128, N], BF16, tag="colacc")
            rowmax_sb = persist.tile([128, NT_X], F32, tag="rowmax_sb")
            colmax_sb = persist.tile([128, NT_Y], F32, tag="colmax_sb")
            rx_sb = persist.tile([128, NT_X], F32, tag="rx_sb")
            ident_bf = persist.tile([128, 128], BF16, tag="ident_bf")
            ident_f32 = persist.tile([128, 128], F32, tag="ident_f32")
            make_identity(nc, ident_bf[:])
            make_identity(nc, ident_f32[:])

            pools = {
                "raw": ep(tc.tile_pool(name="raw", bufs=3)),
                "sq": ep(tc.tile_pool(name="sq", bufs=2)),
                "sc": ep(tc.tile_pool(name="sc", bufs=9)),
                "bf": ep(tc.tile_pool(name="bf", bufs=3)),
                "pps": ep(tc.tile_pool(name="pps", bufs=2, space="PSUM")),
            }
            mm_pool = ep(tc.tile_pool(name="mm", bufs=3, space="PSUM"))
            csb_pool = ep(tc.tile_pool(name="csb", bufs=6))
            row_pool = ep(tc.tile_pool(name="row", bufs=2))
            out_pool = ep(tc.tile_pool(name="out", bufs=2))

            # ---- prep: x (one batch), then y (8 batches) ----
            xv = ex.rearrange("(q p) d -> p q d", p=128)
            _emit_prep_batch(nc, pools, xv, 0, NT_X, xT_hi, xT_lo, ident_bf)
            yv = ey.rearrange("(b q p) d -> b p q d", p=128, q=8)
            for b in range(NB_Y):
                _emit_prep_batch(nc, pools, yv[b], b, 8, yT_hi, yT_lo, ident_bf)

            # ---- matmul sweep + reductions ----
            for xt in range(NT_X):
                rowacc = row_pool.tile([128, JG], BF16, tag="row")
                xs = slice(xt * 128, (xt + 1) * 128)
                for g in range(NG):
                    ps = mm_pool.tile([128, JG], F32, tag="mm")
                    for c in range(JG // 512):
                        j0 = g * JG + c * 512
                        pslice = ps[:, c * 512 : (c + 1) * 512]
                        nc.tensor.matmul(
                            pslice,
                            xT_hi[:, xs],
                            yT_hi[:, j0 : j0 + 512],
                            start=True,
                            stop=False,
                        )
                        nc.tensor.matmul(
                            pslice,
                            xT_lo[:, xs],
                            yT_lo[:, j0 : j0 + 512],
                            start=False,
                            stop=True,
                        )
                    c_sb = csb_pool.tile([128, JG], BF16, tag="csb")
                    nc.scalar.activation(c_sb[:], ps[:], AF.Copy)
                    # col-max accumulate across x-tiles
                    acc_slice = colacc[:, g * JG : (g + 1) * JG]
                    if xt == 0:
                        nc.vector.tensor_copy(acc_slice, c_sb[:])
                    else:
                        nc.vector.tensor_max(acc_slice, acc_slice, c_sb[:])
                    # row-max chain within this x-tile (group width)
                    if g == 0:
                        nc.vector.tensor_copy(rowacc[:], c_sb[:])
                    else:
                        nc.vector.tensor_max(rowacc[:], rowacc[:], c_sb[:])
                nc.vector.reduce_max(
                    rowmax_sb[:, xt : xt + 1], rowacc[:], axis=AX.X
                )

            # ---- col-max partition fold ----
            for fg in range(NT_Y // 4):
                ps = pools["pps"].tile([128, 512], BF16, tag="pps")
                for k in range(4):
                    cch = fg * 4 + k
                    nc.tensor.transpose(
                        ps[:, k * 128 : (k + 1) * 128],
                        colacc[:, cch * 128 : (cch + 1) * 128],
                        ident_bf[:],
                    )
                nc.vector.reduce_max(
                    colmax_sb[:, fg * 4 : (fg + 1) * 4],
                    ps[:].rearrange("p (k q) -> p k q", k=4),
                    axis=AX.X,
                )

            # ---- outputs: transpose on PE so DMA writes are contiguous ----
            pso = pools["pps"].tile([128, 128], F32, tag="pps")
            # rowmax [128, 8] -> [8, 128]
            nc.tensor.transpose(pso[0:8, 0:128], rowmax_sb[:], ident_f32[:])
            rout = out_pool.tile([128, 128], F32, tag="out")
            nc.vector.tensor_copy(rout[0:8, 0:128], pso[0:8, 0:128])
            nc.sync.dma_start(rowmax_o.rearrange("(t p) -> t p", p=128), rout[0:8, :])
            # colmax [128, 64] -> [64, 128]
            pso2 = pools["pps"].tile([128, 128], F32, tag="pps")
            nc.tensor.transpose(pso2[0:64, 0:128], colmax_sb[:], ident_f32[:])
            cout = out_pool.tile([128, 128], F32, tag="out")
            nc.vector.tensor_copy(cout[0:64, 0:128], pso2[0:64, 0:128])
            nc.sync.dma_start(colmax_o.rearrange("(c p) -> c p", p=128), cout[0:64, :])

    _split_multi_waits(nc)
    return nc


_NC_CACHE = []


def _get_nc():
    if not _NC_CACHE:
        _NC_CACHE.append(_build())
    return _NC_CACHE[0]


def run_device(ex, ey, trace=False):
    """Run the SPMD kernel; returns (rowmax [N], colmax [N], results obj)."""
    nc = _get_nc()
    in_maps = [
        {"ex_sh": np.ascontiguousarray(ex[k * XR : (k + 1) * XR]), "ey": ey}
        for k in range(N_CORES)
    ]
    res = bass_utils.run_bass_kernel_spmd(
        nc, in_maps, core_ids=list(range(N_CORES)), trace=trace
    )
    rowmax = np.concatenate([res.results[k]["rowmax"] for k in range(N_CORES)])
    colmax = np.max(
        np.stack([res.results[k]["colmax"] for k in range(N_CORES)]), axis=0
    )
    return rowmax, colmax, res


def _entropy(m):
    # -sum(exp(c)*c), c = logprob_Normal(1,SIGMA)(1 - m); accumulate in f64
    z = -m.astype(np.float64) / SIGMA
    c = -0.5 * z * z - np.log(SIGMA) - 0.5 * np.log(2.0 * np.pi)
    return -np.sum(np.exp(c) * c)


def kernel(ex, ey):
    ex = np.ascontiguousarray(np.asarray(ex), dtype=np.float32)
    ey = np.ascontiguousarray(np.asarray(ey), dtype=np.float32)
    rowmax, colmax, _ = run_device(ex, ey)
    out1 = np.float32(_entropy(rowmax))
    out2 = np.float32(_entropy(colmax))
    return (np.asarray(out1, dtype=np.float32), np.asarray(out2, dtype=np.float32))


# revision 27
# speedup vs baseline: 1.0466x; 1.0097x over previous
"""Trainium2 Bass kernel for nn_DKL_45810121179236 (retrieval_knn).

Reference computation:
    C = cos_sim_matrix(ex, ey)            # [8192, 8192], D=256
    out1 = -sum(exp(c1)*c1), c1 = logN(1 - rowmax(C))
    out2 = -sum(exp(c2)*c2), c2 = logN(1 - colmax(C))

Sharding: ex rows split across 8 NeuronCores (1024 rows each); each core
computes its [1024, 8192] tile of C against the full ey, producing exact
local row-maxes and partial col-maxes. Host combines: concat row-maxes,
8-way elementwise max of col-max partials, then the two O(N) entropy sums.

Per-core pipeline (engines in parens):
  load:   1MB batches, 1024 rows as [128, (8, 256)] (sync HWDGE)
  norm:   per 256-slice sum-of-squares (ACT Square+accum), sqrt (ACT),
          reciprocal (DVE), scale+cast bf16 (ACT Copy w/ scale AP)
  transp: PE transpose 128x128 into PSUM, 4 per bank, copy out (DVE)
  mm:     [128, 1024] PSUM groups; 2x(N=512) x 2 K-chunk bf16 matmuls (PE)
  reduce: PSUM->SBUF bf16 copy (ACT); col-max acc TT-max (DVE, 2x mode);
          row-max chain TT-max (DVE)
  fold:   PE-transpose col-max acc, 3-D reduce_max (DVE)
"""

import sys

sys.path.insert(0, "/opt/trn_rl_repo")

import copy
from contextlib import ExitStack

import numpy as np

import concourse.bass as bass
import concourse.tile as tile
from concourse import mybir
from concourse import bass_utils
from concourse.masks import make_identity

N_CORES = 8
N = 8192  # ey rows (and total ex rows)
D = 256  # embedding dim
XR = N // N_CORES  # ex rows per core (1024)
NT_X = XR // 128  # 8 x-tiles per core
NT_Y = N // 128  # 64 y-tiles
NB_Y = 8  # y load batches (1024 rows each)
JG = 1024  # j-group width = 2 PSUM banks
NG = N // JG  # 8 j-groups per x-tile

F32 = mybir.dt.float32
BF16 = mybir.dt.bfloat16
AF = mybir.ActivationFunctionType
ALU = mybir.AluOpType
AX = mybir.AxisListType

SIGMA = 0.3


def _split_multi_waits(nc, max_waits=1):
    """The walrus build in this container rejects instructions carrying more
    than one sync wait. Move excess waits onto preceding same-engine NOPs
    (waits on one engine are sequential, so semantics are unchanged)."""
    n_split = 0
    for function in nc.m.functions:
        new_blocks = []
        for block in function.blocks:
            new_insts = []
            for inst in block.instructions:
                si = inst.sync_info
                if si is not None and si.on_wait and len(si.on_wait) > max_waits:
                    waits = list(si.on_wait)
                    n_split += 1
                    head, rest = waits[:-max_waits], waits[-max_waits:]
                    for ci in range(0, len(head), max_waits):
                        new_insts.append(
                            mybir.InstNoOp(
                                name=f"{inst.name}-ws{ci}",
                                engine=inst.engine,
                                sync_info=mybir.SyncInfo(
                                    on_wait=head[ci : ci + max_waits], on_update=[]
                                ),
                            )
                        )
                    inst = copy.replace(
                        inst,
                        sync_info=mybir.SyncInfo(
                            on_wait=rest, on_update=list(si.on_update)
                        ),
                    )
                new_insts.append(inst)
            new_blocks.append(copy.replace(block, instructions=new_insts))
        function.blocks.clear()
        for b in new_blocks:
            function.blocks.append(b)
    return n_split


def _emit_prep_batch(nc, pools, src_pqd, b, nq, tT_hi, tT_lo, ident, rns_out=None):
    """Load nq*128 rows, normalize, cast bf16, PE-transpose into tT_hi/lo
    columns [b*128*nq, ...). src_pqd is a [p, q, d] DRAM view of this batch.
    With rns_out, the scale step is skipped (folded downstream) and the
    reciprocal norms are stored there instead."""
    raw = pools["raw"].tile([128, nq * D], F32, tag="raw")
    nc.sync.dma_start(raw[:].rearrange("p (q d) -> p q d", q=nq), src_pqd)

    nsq = pools["sc"].tile([128, nq], F32, tag="sc")
    sq = pools["sq"].tile([128, D], F32, tag="sq")
    for q in range(nq):
        nc.vector.scalar_tensor_tensor(
            sq[:],
            raw[:, q * D : (q + 1) * D],
            1.0,
            raw[:, q * D : (q + 1) * D],
            ALU.mult,
            ALU.mult,
            accum_out=nsq[:, q : q + 1],
        )
    nrm = pools["sc"].tile([128, nq], F32, tag="sc")
    nc.scalar.activation(nrm[:], nsq[:], AF.Sqrt)
    rns = rns_out if rns_out is not None else pools["sc"].tile(
        [128, nq], F32, tag="sc"
    )
    nc.vector.reciprocal(rns[:], nrm[:])

    ybf = pools["bf"].tile([128, nq * D], BF16, tag="bf")
    for q in range(nq):
        if rns_out is None:
            # scale+cast on ACT: out = Copy(in * rns[q])
            nc.scalar.activation(
                ybf[:, q * D : (q + 1) * D],
                raw[:, q * D : (q + 1) * D],
                AF.Copy,
                scale=rns[:, q : q + 1],
            )
        else:
            # x side: plain cast; 1/||x_i|| folds into the PSUM->SBUF copies
            nc.scalar.activation(
                ybf[:, q * D : (q + 1) * D], raw[:, q * D : (q + 1) * D], AF.Copy
            )

    # PE transposes: 4 per [128, 512] psum tile, then one copy per tile
    for half, tT in ((0, tT_hi), (1, tT_lo)):
        for qq in range(0, nq, 4):
            ps = pools["pps"].tile([128, 512], BF16, tag="pps")
            for q in range(qq, min(qq + 4, nq)):
                nc.tensor.transpose(
                    ps[:, (q - qq) * 128 : (q - qq + 1) * 128],
                    ybf[:, q * D + half * 128 : q * D + half * 128 + 128],
                    ident[:],
                )
            w = (min(qq + 4, nq) - qq) * 128
            col0 = b * nq * 128 + qq * 128
            nc.vector.tensor_copy(tT[:, col0 : col0 + w], ps[:, 0:w])


def _build():
    nc = bass.Bass("TRN2", target_bir_lowering=False, debug=False, num_devices=1)
    ex = nc.dram_tensor("ex_sh", [XR, D], F32, kind="ExternalInput").ap()
    ey = nc.dram_tensor("ey", [N, D], F32, kind="ExternalInput").ap()
    rowmax_o = nc.dram_tensor("rowmax", [XR], F32, kind="ExternalOutput").ap()
    colmax_o = nc.dram_tensor("colmax", [N], F32, kind="ExternalOutput").ap()

    with tile.TileContext(nc) as tc:
        with ExitStack() as ctx:
            ep = ctx.enter_context

            persist = ep(tc.tile_pool(name="persist", bufs=1))
            yT_hi = persist.tile([128, N], BF16, tag="yT_hi")
            yT_lo = persist.tile([128, N], BF16, tag="yT_lo")
            xT_hi = persist.tile([128, XR], BF16, tag="xT_hi")
            xT_lo = persist.tile([128, XR], BF16, tag="xT_lo")
            colacc = persist.tile([128, N], BF16, tag="colacc")
            rowmax_sb = persist.tile([128, NT_X], F32, tag="rowmax_sb")
            colmax_sb = persist.tile([128, NT_Y], F32, tag="colmax_sb")
            rx_sb = persist.tile([128, NT_X], F32, tag="rx_sb")
            ident_bf = persist.tile([128, 128], BF16, tag="ident_bf")
            ident_f32 = persist.tile([128, 128], F32, tag="ident_f32")
            make_identity(nc, ident_bf[:])
            make_identity(nc, ident_f32[:])

            pools = {
                "raw": ep(tc.tile_pool(name="raw", bufs=3)),
                "sq": ep(tc.tile_pool(name="sq", bufs=2)),
                "sc": ep(tc.tile_pool(name="sc", bufs=9)),
                "bf": ep(tc.tile_pool(name="bf", bufs=3)),
                "pps": ep(tc.tile_pool(name="pps", bufs=2, space="PSUM")),
            }
            mm_pool = ep(tc.tile_pool(name="mm", bufs=3, space="PSUM"))
            csb_pool = ep(tc.tile_pool(name="csb", bufs=4))
            row_pool = ep(tc.tile_pool(name="row", bufs=2))
            out_pool = ep(tc.tile_pool(name="out", bufs=2))

            # ---- prep: x (one batch), then y (8 batches) ----
            xv = ex.rearrange("(q p) d -> p q d", p=128)
            _emit_prep_batch(nc, pools, xv, 0, NT_X, xT_hi, xT_lo, ident_bf)
            yv = ey.rearrange("(b q p) d -> b p q d", p=128, q=8)
            for b in range(NB_Y):
                _emit_prep_batch(nc, pools, yv[b], b, 8, yT_hi, yT_lo, ident_bf)

            # ---- matmul sweep + reductions ----
            for xt in range(NT_X):
                rowacc = row_pool.tile([128, JG], BF16, tag="row")
                xs = slice(xt * 128, (xt + 1) * 128)
                for g in range(NG):
                    ps = mm_pool.tile([128, JG], F32, tag="mm")
                    for c in range(JG // 512):
                        j0 = g * JG + c * 512
                        pslice = ps[:, c * 512 : (c + 1) * 512]
                        nc.tensor.matmul(
                            pslice,
                            xT_hi[:, xs],
                            yT_hi[:, j0 : j0 + 512],
                            start=True,
                            stop=False,
                        )
                        nc.tensor.matmul(
                            pslice,
                            xT_lo[:, xs],
                            yT_lo[:, j0 : j0 + 512],
                            start=False,
                            stop=True,
                        )
                    c_sb = csb_pool.tile([128, JG], BF16, tag="csb")
                    nc.scalar.activation(c_sb[:], ps[:], AF.Copy)
                    # col-max accumulate across x-tiles
                    acc_slice = colacc[:, g * JG : (g + 1) * JG]
                    if xt == 0:
                        nc.vector.tensor_copy(acc_slice, c_sb[:])
                    else:
                        nc.vector.tensor_max(acc_slice, acc_slice, c_sb[:])
                    # row-max chain within this x-tile (group width)
                    if g == 0:
                        nc.vector.tensor_copy(rowacc[:], c_sb[:])
                    else:
                        nc.vector.tensor_max(rowacc[:], rowacc[:], c_sb[:])
                nc.vector.reduce_max(
                    rowmax_sb[:, xt : xt + 1], rowacc[:], axis=AX.X
                )

            # ---- col-max partition fold ----
            for fg in range(NT_Y // 4):
                ps = pools["pps"].tile([128, 512], BF16, tag="pps")
                for k in range(4):
                    cch = fg * 4 + k
                    nc.tensor.transpose(
                        ps[:, k * 128 : (k + 1) * 128],
                        colacc[:, cch * 128 : (cch + 1) * 128],
                        ident_bf[:],
                    )
                nc.vector.reduce_max(
                    colmax_sb[:, fg * 4 : (fg + 1) * 4],
                    ps[:].rearrange("p (k q) -> p k q", k=4),
                    axis=AX.X,
                )

            # ---- outputs: transpose on PE so DMA writes are contiguous ----
            pso = pools["pps"].tile([128, 128], F32, tag="pps")
            # rowmax [128, 8] -> [8, 128]
            nc.tensor.transpose(pso[0:8, 0:128], rowmax_sb[:], ident_f32[:])
            rout = out_pool.tile([128, 128], F32, tag="out")
            nc.vector.tensor_copy(rout[0:8, 0:128], pso[0:8, 0:128])
            nc.sync.dma_start(rowmax_o.rearrange("(t p) -> t p", p=128), rout[0:8, :])
            # colmax [128, 64] -> [64, 128]
            pso2 = pools["pps"].tile([128, 128], F32, tag="pps")
            nc.tensor.transpose(pso2[0:64, 0:128], colmax_sb[:], ident_f32[:])
            cout = out_pool.tile([128, 128], F32, tag="out")
            nc.vector.tensor_copy(cout[0:64, 0:128], pso2[0:64, 0:128])
            nc.sync.dma_start(colmax_o.rearrange("(c p) -> c p", p=128), cout[0:64, :])

    _split_multi_waits(nc)
    return nc


_NC_CACHE = []


def _get_nc():
    if not _NC_CACHE:
        _NC_CACHE.append(_build())
    return _NC_CACHE[0]


def run_device(ex, ey, trace=False):
    """Run the SPMD kernel; returns (rowmax [N], colmax [N], results obj)."""
    nc = _get_nc()
    in_maps = [
        {"ex_sh": np.ascontiguousarray(ex[k * XR : (k + 1) * XR]), "ey": ey}
        for k in range(N_CORES)
    ]
    res = bass_utils.run_bass_kernel_spmd(
        nc, in_maps, core_ids=list(range(N_CORES)), trace=trace
    )
    rowmax = np.concatenate([res.results[k]["rowmax"] for k in range(N_CORES)])
    colmax = np.max(
        np.stack([res.results[k]["colmax"] for k in range(N_CORES)]), axis=0
    )
    return rowmax, colmax, res


def _entropy(m):
    # -sum(exp(c)*c), c = logprob_Normal(1,SIGMA)(1 - m); accumulate in f64
    z = -m.astype(np.float64) / SIGMA
    c = -0.5 * z * z - np.log(SIGMA) - 0.5 * np.log(2.0 * np.pi)
    return -np.sum(np.exp(c) * c)


def kernel(ex, ey):
    ex = np.ascontiguousarray(np.asarray(ex), dtype=np.float32)
    ey = np.ascontiguousarray(np.asarray(ey), dtype=np.float32)
    rowmax, colmax, _ = run_device(ex, ey)
    out1 = np.float32(_entropy(rowmax))
    out2 = np.float32(_entropy(colmax))
    return (np.asarray(out1, dtype=np.float32), np.asarray(out2, dtype=np.float32))
